# revision 19
# baseline (speedup 1.0000x reference)
"""Soft-KNN Bass/Tile kernel for Trainium2 (8 NeuronCores, axon/PJRT).

Strategy (v2)
-------------
- Host-side prep (no device setup phase): per core, the 6250-row train shard
  is sorted by label; host emits pre-transposed operand tensors:
    * hi terms in fp16 at PSUM scale 512:  PSUM = Xh@Yh^T + cross - 512*yn,
      X = 1024*x, Xh = fp16(X), Yh = fp16(y).
    * cross terms in fp8e4m3 with DoubleRow interleave (2 contraction rows
      per partition, 2x PE rate): plane j=0 = (e4m3(Xh/512), e4m3(512*Yl)),
      plane j=1 = (e4m3(Xl), e4m3(Yh)).
    * yn as a 3-row bf16 ladder of 512*yn (pad cols get +1e30 -> z=-inf).
  All operands stay SBUF-resident (~17MB/core); no streaming in main loop.
- Main loop per (group of 4 qtiles, 2048-col window, qtile): 9 matmuls per
  512-chunk accumulate 512*z into a [128,2048] PSUM tile; DVE max8 +
  find_index8 per window -> 8 candidates; 4 windows -> 32 candidates.
  (top-8 per 2048-window is safe: losing a true global-top-16 member needs
  >=9 of them in one window, P ~ 1e-7.)
- Local merge 32 -> exact top-16 (max8/match_replace marking + cumsum-rank
  + gpsimd.local_scatter compaction). Labels via Sign-activation boundary
  counting on the Act engine (sum of sign(idx+0.5-bnd_k) = 2*label-98).
- 4 per-group AllGathers ([512,32] f32 each) overlap with later groups'
  compute. Owner of qtile qt is core qt%8; owner merges 128 candidates to
  global top-16 after AG_1 (qt=pid) and AG_3 (qt=pid+8), then computes
  softmax(-sqrt(xn - z)) and scatter-adds into 100 classes.
- Output per core: [256, 100] (rows of qtiles pid and pid+8).
"""

import numpy as np
import ml_dtypes

import concourse.bass as bass
import concourse.bacc as bacc
import concourse.mybir as mybir
import concourse.tile as tile
from concourse import bass_utils

F32 = mybir.dt.float32
F16 = mybir.dt.float16
BF16 = mybir.dt.bfloat16
F8E4 = mybir.dt.float8e4
U8 = mybir.dt.uint8
U16 = mybir.dt.uint16
I16 = mybir.dt.int16
I32 = mybir.dt.int32
AL = mybir.AluOpType
AF = mybir.ActivationFunctionType
DR = mybir.MatmulPerfMode.DoubleRow

NCORES = 8
B = 2048                  # queries
D = 512                   # feature dim
NSHARD = 6250             # train rows per core
COLS = 6272               # padded columns
WIN = [(0, 2048), (2048, 4096), (4096, 6144), (6144, 6272)]
NW = len(WIN)
NCAND = 8 * NW            # 32 candidates per qtile per core
QTILES = B // 128         # 16
GROUPS = 4
GQT = QTILES // GROUPS    # 4
NCLASS = 100
K = 16
NG = NCORES * K           # 128
NEG = -3.0e38
BIG = 1.0e30
BETA = 512.0              # PSUM scale


def _merge_top16(nc, small, uniq, vals, width, payloads):
    """Exact top-16 of `vals` [128, width] via max8/match_replace marking +
    cumsum-rank compaction. `payloads`: list of (ap_u16_plane, out_tile)
    compacted with gpsimd.local_scatter in descending-value order."""
    t8a = small.tile([128, 8], F32, name=f"{uniq}_t8a", tag="mg_t8a")
    t8b = small.tile([128, 8], F32, name=f"{uniq}_t8b", tag="mg_t8b")
    m1 = small.tile([128, NG], F32, name=f"{uniq}_m1", tag="mg_m1")
    m2 = small.tile([128, NG], F32, name=f"{uniq}_m2", tag="mg_m2")
    nc.vector.max(t8a[:], vals[:, :width])
    nc.vector.match_replace(m1[:, :width], t8a[:], vals[:, :width], NEG)
    nc.vector.max(t8b[:], m1[:, :width])
    nc.vector.match_replace(m2[:, :width], t8b[:], m1[:, :width], NEG)
    mask = small.tile([128, NG], F32, name=f"{uniq}_mask", tag="mg_mask")
    nc.vector.tensor_scalar(out=mask[:, :width], in0=m2[:, :width],
                            scalar1=-2e38, scalar2=None, op0=AL.is_le)
    csA = small.tile([128, NG], F32, name=f"{uniq}_csA", tag="mg_csA")
    csB = small.tile([128, NG], F32, name=f"{uniq}_csB", tag="mg_csB")
    nc.vector.tensor_copy(csA[:, :width], mask[:, :width])
    src, dst = csA, csB
    sh = 1
    while sh < width:
        nc.vector.tensor_copy(dst[:, 0:sh], src[:, 0:sh])
        nc.vector.tensor_tensor(out=dst[:, sh:width], in0=src[:, sh:width],
                                in1=src[:, 0:width - sh], op=AL.add)
        src, dst = dst, src
        sh *= 2
    rk = small.tile([128, NG], F32, name=f"{uniq}_rk", tag="mg_rk")
    nc.vector.tensor_tensor(out=rk[:, :width], in0=src[:, :width],
                            in1=mask[:, :width], op=AL.mult)
    nc.vector.tensor_scalar(out=rk[:, :width], in0=rk[:, :width], scalar1=-1.0,
                            scalar2=None, op0=AL.add)
    rk16 = small.tile([128, NG], I16, name=f"{uniq}_rk16", tag="mg_rk16")
    nc.vector.tensor_copy(rk16[:, :width], rk[:, :width])
    for plane, out16 in payloads:
        nc.gpsimd.local_scatter(out16[:].bitcast(I16), plane.bitcast(I16),
                                rk16[:, :width], channels=128, num_elems=K,
                                num_idxs=width)


def build():
    nc = bacc.Bacc("TRN2", target_bir_lowering=False, num_devices=NCORES)

    xh_in = nc.dram_tensor("xh", [4, 128, B], U16, kind="ExternalInput")
    xc8_in = nc.dram_tensor("xc8", [4, 128, 2, B], U8, kind="ExternalInput")
    yh_in = nc.dram_tensor("yh", [4, 128, COLS], U16, kind="ExternalInput")
    yc8_in = nc.dram_tensor("yc8", [4, 128, 2, COLS], U8, kind="ExternalInput")
    yn3_in = nc.dram_tensor("yn3", [3, COLS], U16, kind="ExternalInput")
    xn_in = nc.dram_tensor("xn", [128, QTILES], F32, kind="ExternalInput")
    bnd_in = nc.dram_tensor("bnd", [1, NCLASS], F32, kind="ExternalInput")
    out_d = nc.dram_tensor("out", [2 * 128, NCLASS], F32,
                           kind="ExternalOutput")

    ag_in = [nc.dram_tensor(f"ag_in{g}", [GQT * 128, 2 * K], F32)
             for g in range(GROUPS)]
    ag_out = nc.dram_tensor("ag_out", [GROUPS * NCORES * GQT * 128, 2 * K],
                            F32, addr_space="Shared")

    with tile.TileContext(nc) as tc:
        with tc.tile_pool(name="res", bufs=1) as res, \
             tc.tile_pool(name="zps", bufs=2, space="PSUM") as zps, \
             tc.tile_pool(name="candp", bufs=2) as candp, \
             tc.tile_pool(name="small", bufs=2) as small:

            # ---------------- resident tensors ----------------
            xh_r = [res.tile([128, B], F16, name=f"xh{k}") for k in range(4)]
            xc8_r = [res.tile([128, 2, B], F8E4, name=f"xc{k}")
                     for k in range(4)]
            yh_r = [res.tile([128, COLS], F16, name=f"yh{k}") for k in range(4)]
            yc8_r = [res.tile([128, 2, COLS], F8E4, name=f"yc{k}")
                     for k in range(4)]
            yn3 = res.tile([3, COLS], BF16)
            ones3 = res.tile([3, 128], BF16)
            xn_r = res.tile([128, QTILES], F32)
            bnd_f = res.tile([128, NCLASS], F32)
            cio_f = res.tile([128, NCLASS], F32)
            base32 = res.tile([128, NCAND], U16)

            # small constants first
            nc.vector.memset(ones3[:], -1.0)
            nc.sync.dma_start(yn3[:].bitcast(U16), yn3_in[:])
            nc.sync.dma_start(xn_r[:], xn_in[:])
            nc.gpsimd.iota(base32[:], pattern=[[2048, NW], [0, 8]],
                           channel_multiplier=0)
            cio_i = small.tile([128, NCLASS], I32, tag="cioi", bufs=1)
            nc.gpsimd.iota(cio_i[:], pattern=[[1, NCLASS]],
                           channel_multiplier=0)
            nc.vector.tensor_copy(cio_f[:], cio_i[:])
            # broadcast bnd row to 128 partitions via f32 matmul
            bnd_row = small.tile([1, NCLASS], F32, tag="bndrow", bufs=1)
            nc.sync.dma_start(bnd_row[:], bnd_in[:])
            ones1 = small.tile([1, 128], F32, tag="ones1", bufs=1)
            nc.vector.memset(ones1[:], 1.0)
            bps = zps.tile([128, 2048], F32, name="bps", tag="zw")
            nc.tensor.matmul(bps[:, 0:NCLASS], ones1[:], bnd_row[:],
                             start=True, stop=True)
            nc.scalar.copy(bnd_f[:], bps[:, 0:NCLASS])

            # x side (needed by every window)
            for k in range(4):
                nc.sync.dma_start(xh_r[k][:].bitcast(U16), xh_in[k])
                nc.sync.dma_start(xc8_r[k][:].bitcast(U8), xc8_in[k])
            # y side in column blocks so window 0 can start early
            for (a, b) in WIN:
                for k in range(4):
                    nc.sync.dma_start(yh_r[k][:, a:b].bitcast(U16),
                                      yh_in[k, :, a:b])
                    nc.sync.dma_start(yc8_r[k][:, :, a:b].bitcast(U8),
                                      yc8_in[k, :, :, a:b])

            pid_sp = nc.sync.partition_id()

            def global_phase(l):
                """Merge + vote for owned qtile qt = pid + 8*l."""
                gv = small.tile([128, NG], F32, name=f"gv{l}", tag="gv")
                gl = small.tile([128, NG], F32, name=f"gl{l}", tag="gl")
                gvl3 = small.tile([128, NCORES, 2 * K], F32,
                                  name=f"gvl{l}", tag="gvl")
                g_reg = (pid_sp + 8 * l) // 4
                lq_reg = pid_sp % 4
                for c2 in range(NCORES):
                    base = g_reg * (NCORES * GQT * 128) + c2 * (GQT * 128)
                    nc.sync.dma_start(
                        gvl3[:, c2, :],
                        ag_out[bass.ds(base + lq_reg * 128, 128), :])
                nc.vector.tensor_copy(
                    gv[:].rearrange("p (c k) -> p c k", c=NCORES),
                    gvl3[:, :, 0:K])
                nc.vector.tensor_copy(
                    gl[:].rearrange("p (c k) -> p c k", c=NCORES),
                    gvl3[:, :, K:2 * K])
                vlo = small.tile([128, NG], U16, name=f"gvlo{l}", tag="vlo")
                vhi = small.tile([128, NG], U16, name=f"gvhi{l}", tag="vhi")
                gvu = gv[:].bitcast(U16).rearrange("p (a two) -> p a two",
                                                   two=2)
                nc.vector.tensor_copy(vlo[:], gvu[:, :, 0:1])
                nc.vector.tensor_copy(vhi[:], gvu[:, :, 1:2])
                glu = small.tile([128, NG], U16, name=f"glu{l}", tag="glu")
                nc.vector.tensor_copy(glu[:], gl[:])
                slo = small.tile([128, K], U16, name=f"gslo{l}", tag="slo16")
                shi = small.tile([128, K], U16, name=f"gshi{l}", tag="shi16")
                sla = small.tile([128, K], U16, name=f"gsla{l}", tag="sgi16")
                _merge_top16(nc, small, f"gm{l}", gv, NG,
                             [(vlo[:], slo), (vhi[:], shi), (glu[:], sla)])
                v16 = small.tile([128, K], F32, name=f"gv16{l}", tag="v16")
                v16u = v16[:].bitcast(U16).rearrange("p (a two) -> p a two",
                                                     two=2)
                nc.vector.tensor_copy(v16u[:, :, 0:1], slo[:])
                nc.vector.tensor_copy(v16u[:, :, 1:2], shi[:])
                lab16 = small.tile([128, K], F32, name=f"glab{l}",
                                   tag="lab16")
                nc.vector.tensor_copy(lab16[:], sla[:])
                xn_col = small.tile([128, 1], F32, name=f"xnc{l}",
                                    tag="xncol")
                nc.sync.dma_start(xn_col[:],
                                  xn_r[:, bass.ds(pid_sp + 8 * l, 1)])
                dsq = small.tile([128, K], F32, name=f"dsq{l}", tag="dsq")
                nc.scalar.activation(dsq[:], v16[:], AF.Sqrt,
                                     scale=-1.0 / BETA, bias=xn_col[:, 0:1])
                ew = small.tile([128, K], F32, name=f"ew{l}", tag="ew")
                zsum = small.tile([128, 1], F32, name=f"zs{l}", tag="zs")
                nc.scalar.activation(ew[:], dsq[:], AF.Exp, scale=-1.0,
                                     accum_out=zsum[:, 0:1])
                rz = small.tile([128, 1], F32, name=f"rz{l}", tag="rz")
                nc.vector.reciprocal(rz[:], zsum[:])
                wt = small.tile([128, K], F32, name=f"wt{l}", tag="wt")
                nc.vector.tensor_scalar(out=wt[:], in0=ew[:],
                                        scalar1=rz[:, 0:1], scalar2=None,
                                        op0=AL.mult)
                vote = small.tile([128, NCLASS], F32, name=f"vote{l}",
                                  tag="vote")
                voteg = small.tile([128, NCLASS], F32, name=f"voteg{l}",
                                   tag="voteg")
                tmp = small.tile([128, NCLASS], F32, name=f"vtmp{l}",
                                 tag="vtmp")
                tmpg = small.tile([128, NCLASS], F32, name=f"vtmpg{l}",
                                  tag="vtmpg")
                nc.vector.memset(vote[:], 0.0)
                nc.gpsimd.memset(voteg[:], 0.0)
                for r in range(K // 2):
                    nc.vector.tensor_scalar(out=tmp[:], in0=cio_f[:],
                                            scalar1=lab16[:, r:r + 1],
                                            scalar2=wt[:, r:r + 1],
                                            op0=AL.is_equal, op1=AL.mult)
                    nc.vector.tensor_tensor(out=vote[:], in0=vote[:],
                                            in1=tmp[:], op=AL.add)
                    r2 = r + K // 2
                    nc.gpsimd.tensor_scalar(out=tmpg[:], in0=cio_f[:],
                                            scalar1=lab16[:, r2:r2 + 1],
                                            scalar2=wt[:, r2:r2 + 1],
                                            op0=AL.is_equal, op1=AL.mult)
                    nc.gpsimd.tensor_tensor(out=voteg[:], in0=voteg[:],
                                            in1=tmpg[:], op=AL.add)
                nc.vector.tensor_tensor(out=vote[:], in0=vote[:],
                                        in1=voteg[:], op=AL.add)
                nc.sync.dma_start(out_d[l * 128:(l + 1) * 128, :], vote[:])

            def merge_qtile(g, lq, cv, ci):
                """Exact top-16 + labels for one qtile, right after its
                last window scan (spreads DVE/Act bursts across the group)."""
                qt = g * GQT + lq
                gi = small.tile([128, NCAND], U16, name=f"gi{qt}",
                                tag="gi")
                nc.vector.tensor_tensor(out=gi[:], in0=ci[:],
                                        in1=base32[:], op=AL.add)
                vlo = small.tile([128, NCAND], U16, name=f"vlo{qt}",
                                 tag="vlo")
                vhi = small.tile([128, NCAND], U16, name=f"vhi{qt}",
                                 tag="vhi")
                cvu = cv[:].bitcast(U16).rearrange(
                    "p (a two) -> p a two", two=2)
                nc.vector.tensor_copy(vlo[:, :NCAND], cvu[:, :, 0:1])
                nc.vector.tensor_copy(vhi[:, :NCAND], cvu[:, :, 1:2])
                slo = small.tile([128, K], U16, name=f"slo16_{qt}",
                                 tag="slo16")
                shi = small.tile([128, K], U16, name=f"shi16_{qt}",
                                 tag="shi16")
                sgi = small.tile([128, K], U16, name=f"sgi16_{qt}",
                                 tag="sgi16")
                _merge_top16(nc, small, f"lm{qt}", cv, NCAND,
                             [(vlo[:, :NCAND], slo),
                              (vhi[:, :NCAND], shi), (gi[:], sgi)])
                v16 = small.tile([128, K], F32, name=f"v16_{qt}",
                                 tag="v16")
                v16u = v16[:].bitcast(U16).rearrange(
                    "p (a two) -> p a two", two=2)
                nc.vector.tensor_copy(v16u[:, :, 0:1], slo[:])
                nc.vector.tensor_copy(v16u[:, :, 1:2], shi[:])
                # labels: sum of sign(idx + 0.5 - bnd_k) = 2*label-98
                gfh = small.tile([128, K], F32, name=f"gfh{qt}",
                                 tag="gfh")
                nc.vector.tensor_scalar(out=gfh[:], in0=sgi[:],
                                        scalar1=0.5, scalar2=None,
                                        op0=AL.add)
                junk = small.tile([128, NCLASS], F32, name=f"sj{qt}",
                                  tag="sjunk")
                labsum = small.tile([128, K], F32, name=f"ls{qt}",
                                    tag="labsum")
                for r in range(K):
                    nc.scalar.activation(junk[:], bnd_f[:], AF.Sign,
                                         bias=gfh[:, r:r + 1],
                                         scale=-1.0,
                                         accum_out=labsum[:, r:r + 1])
                lab16 = small.tile([128, K], F32, name=f"lab{qt}",
                                   tag="lab16l")
                nc.vector.tensor_scalar(out=lab16[:], in0=labsum[:],
                                        scalar1=0.5, scalar2=49.0,
                                        op0=AL.mult, op1=AL.add)
                nc.sync.dma_start(
                    ag_in[g][lq * 128:(lq + 1) * 128, 0:K], v16[:])
                nc.sync.dma_start(
                    ag_in[g][lq * 128:(lq + 1) * 128, K:2 * K], lab16[:])

            # ---------------- main loop ----------------
            for g in range(GROUPS):
                cands = {}
                for lq in range(GQT):
                    cands[lq] = (
                        candp.tile([128, NCAND], F32, name=f"cv{g}_{lq}",
                                   tag=f"cv{lq}"),
                        candp.tile([128, NCAND], U16, name=f"ci{g}_{lq}",
                                   tag=f"ci{lq}"),
                    )
                for w, (a, b) in enumerate(WIN):
                    wlen = b - a
                    for lq in range(GQT):
                        qt = g * GQT + lq
                        qs = qt * 128
                        ps = zps.tile([128, 2048], F32,
                                      name=f"ps{g}_{w}_{lq}", tag="zw")
                        for co0 in range(0, wlen, 512):
                            co = a + co0
                            cw = min(512, b - co)
                            pslice = ps[:, co0:co0 + cw]
                            nc.tensor.matmul(pslice, ones3[:],
                                             yn3[:, co:co + cw],
                                             start=True, stop=False)
                            for k in range(4):
                                nc.tensor.matmul(pslice,
                                                 xh_r[k][:, qs:qs + 128],
                                                 yh_r[k][:, co:co + cw],
                                                 start=False, stop=False)
                            for k in range(4):
                                nc.tensor.matmul(
                                    pslice,
                                    xc8_r[k][:, :, qs:qs + 128],
                                    yc8_r[k][:, :, co:co + cw],
                                    start=False, stop=(k == 3),
                                    perf_mode=DR)
                        cv, ci = cands[lq]
                        nc.vector.max(cv[:, w * 8:w * 8 + 8], ps[:, :wlen])
                        nc.vector.max_index(ci[:, w * 8:w * 8 + 8],
                                            cv[:, w * 8:w * 8 + 8],
                                            ps[:, :wlen])
                        if w == NW - 1:
                            merge_qtile(g, lq, cv, ci)

                # owned qtile qt=pid sits in group 0 or 1; AG_1 has had all
                # of group 2's scans to land, so this inserts no engine wait
                if g == 2:
                    global_phase(0)

                nc.gpsimd.collective_compute(
                    "AllGather", AL.bypass,
                    replica_groups=[list(range(NCORES))],
                    ins=[ag_in[g][:].opt()],
                    outs=[ag_out[g * NCORES * GQT * 128:
                                 (g + 1) * NCORES * GQT * 128, :].opt()])

                if g == 3:
                    global_phase(1)

    nc.finalize()
    return nc


_NC_CACHE = None


def _e4m3(a):
    return np.clip(a, -240.0, 240.0).astype(ml_dtypes.float8_e4m3fn)


def _prep_host(x, tf, tl):
    """Build per-core input maps (host-side marshalling)."""
    x = np.ascontiguousarray(np.asarray(x, dtype=np.float32))
    tf = np.ascontiguousarray(np.asarray(tf, dtype=np.float32))
    tl = np.asarray(tl, dtype=np.int64)

    X = 1024.0 * x
    Xh16 = X.astype(np.float16)
    Xh = Xh16.astype(np.float32)
    Xl = X - Xh
    A_lhs = _e4m3(Xh / 512.0)
    B_lhs = _e4m3(Xl)

    xh_t = np.ascontiguousarray(
        Xh16.T.reshape(4, 128, B).view(np.uint16))
    xc8 = np.ascontiguousarray(np.stack(
        [A_lhs.T.reshape(4, 128, B).view(np.uint8),
         B_lhs.T.reshape(4, 128, B).view(np.uint8)],
        axis=2))

    xn = (x * x).sum(1).astype(np.float32).reshape(QTILES, 128).T
    xn = np.ascontiguousarray(xn)

    in_maps = []
    for c in range(NCORES):
        sl = slice(c * NSHARD, (c + 1) * NSHARD)
        labs = tl[sl]
        feats = tf[sl]
        perm = np.argsort(labs, kind="stable")
        feats = np.ascontiguousarray(feats[perm])
        labs_s = labs[perm]
        bnd = np.searchsorted(labs_s, np.arange(NCLASS), side="left")

        y = np.zeros((COLS, D), np.float32)
        y[:NSHARD] = feats
        yn = np.full(COLS, BIG, np.float32)
        yn[:NSHARD] = (feats * feats).sum(1)

        Yh16 = y.astype(np.float16)
        Yh = Yh16.astype(np.float32)
        Yl = y - Yh
        A_rhs = _e4m3(512.0 * Yl)
        B_rhs = _e4m3(Yh)

        yh_t = np.ascontiguousarray(
            Yh16.T.reshape(4, 128, COLS).view(np.uint16))
        yc8 = np.ascontiguousarray(np.stack(
            [A_rhs.T.reshape(4, 128, COLS).view(np.uint8),
             B_rhs.T.reshape(4, 128, COLS).view(np.uint8)],
            axis=2))

        v = (BETA * yn).astype(np.float32)
        y1 = v.astype(ml_dtypes.bfloat16)
        y2 = (v - y1.astype(np.float32)).astype(ml_dtypes.bfloat16)
        y3 = (v - y1.astype(np.float32) - y2.astype(np.float32)).astype(
            ml_dtypes.bfloat16)
        yn3 = np.ascontiguousarray(
            np.stack([y1, y2, y3], axis=0).view(np.uint16))

        in_maps.append({
            "xh": xh_t,
            "xc8": xc8,
            "yh": yh_t,
            "yc8": yc8,
            "yn3": yn3,
            "xn": xn,
            "bnd": bnd.astype(np.float32)[None, :],
        })
    return in_maps


def kernel(x, train_features, train_labels, **run_kwargs):
    global _NC_CACHE
    in_maps = _prep_host(x, train_features, train_labels)
    if _NC_CACHE is None:
        _NC_CACHE = build()
    res = bass_utils.run_bass_kernel_spmd(
        _NC_CACHE, in_maps, core_ids=list(range(NCORES)), **run_kwargs)
    global LAST_RESULTS
    LAST_RESULTS = res
    out = np.zeros((B, NCLASS), np.float32)
    for c in range(NCORES):
        o = res.results[c]["out"]
        out[c * 128:(c + 1) * 128] = o[0:128]
        out[(8 + c) * 128:(9 + c) * 128] = o[128:256]
    return out.astype(np.float32)


LAST_RESULTS = None


# revision 22
# speedup vs baseline: 1.0033x; 1.0033x over previous
"""Soft-KNN Bass/Tile kernel for Trainium2 (8 NeuronCores, axon/PJRT).

Strategy (v2)
-------------
- Host-side prep (no device setup phase): per core, the 6250-row train shard
  is sorted by label; host emits pre-transposed operand tensors:
    * hi terms in fp16 at PSUM scale 512:  PSUM = Xh@Yh^T + cross - 512*yn,
      X = 1024*x, Xh = fp16(X), Yh = fp16(y).
    * cross terms in fp8e4m3 with DoubleRow interleave (2 contraction rows
      per partition, 2x PE rate): plane j=0 = (e4m3(Xh/512), e4m3(512*Yl)),
      plane j=1 = (e4m3(Xl), e4m3(Yh)).
    * yn as a 3-row bf16 ladder of 512*yn (pad cols get +1e30 -> z=-inf).
  All operands stay SBUF-resident (~17MB/core); no streaming in main loop.
- Main loop per (group of 4 qtiles, 2048-col window, qtile): 9 matmuls per
  512-chunk accumulate 512*z into a [128,2048] PSUM tile; DVE max8 +
  find_index8 per window -> 8 candidates; 4 windows -> 32 candidates.
  (top-8 per 2048-window is safe: losing a true global-top-16 member needs
  >=9 of them in one window, P ~ 1e-7.)
- Local merge 32 -> exact top-16 (max8/match_replace marking + cumsum-rank
  + gpsimd.local_scatter compaction). Labels via Sign-activation boundary
  counting on the Act engine (sum of sign(idx+0.5-bnd_k) = 2*label-98).
- 4 per-group AllGathers ([512,32] f32 each) overlap with later groups'
  compute. Owner of qtile qt is core qt%8; owner merges 128 candidates to
  global top-16 after AG_1 (qt=pid) and AG_3 (qt=pid+8), then computes
  softmax(-sqrt(xn - z)) and scatter-adds into 100 classes.
- Output per core: [256, 100] (rows of qtiles pid and pid+8).
"""

import numpy as np
import ml_dtypes

import concourse.bass as bass
import concourse.bacc as bacc
import concourse.mybir as mybir
import concourse.tile as tile
from concourse import bass_utils

F32 = mybir.dt.float32
F16 = mybir.dt.float16
BF16 = mybir.dt.bfloat16
F8E4 = mybir.dt.float8e4
U8 = mybir.dt.uint8
U16 = mybir.dt.uint16
I16 = mybir.dt.int16
I32 = mybir.dt.int32
AL = mybir.AluOpType
AF = mybir.ActivationFunctionType
DR = mybir.MatmulPerfMode.DoubleRow

NCORES = 8
B = 2048                  # queries
D = 512                   # feature dim
NSHARD = 6250             # train rows per core
COLS = 6272               # padded columns
WIN = [(0, 2048), (2048, 4096), (4096, 6144), (6144, 6272)]
NW = len(WIN)
NCAND = 8 * NW            # 32 candidates per qtile per core
QTILES = B // 128         # 16
GROUPS = 4
GQT = QTILES // GROUPS    # 4
NCLASS = 100
K = 16
NG = NCORES * K           # 128
NEG = -3.0e38
BIG = 1.0e30
BETA = 512.0              # PSUM scale


def _merge_top16(nc, small, uniq, vals, width, payloads):
    """Exact top-16 of `vals` [128, width] via max8/match_replace marking +
    cumsum-rank compaction. `payloads`: list of (ap_u16_plane, out_tile)
    compacted with gpsimd.local_scatter in descending-value order."""
    t8a = small.tile([128, 8], F32, name=f"{uniq}_t8a", tag="mg_t8a")
    t8b = small.tile([128, 8], F32, name=f"{uniq}_t8b", tag="mg_t8b")
    m1 = small.tile([128, NG], F32, name=f"{uniq}_m1", tag="mg_m1")
    m2 = small.tile([128, NG], F32, name=f"{uniq}_m2", tag="mg_m2")
    nc.vector.max(t8a[:], vals[:, :width])
    nc.vector.match_replace(m1[:, :width], t8a[:], vals[:, :width], NEG)
    nc.vector.max(t8b[:], m1[:, :width])
    nc.vector.match_replace(m2[:, :width], t8b[:], m1[:, :width], NEG)
    mask = small.tile([128, NG], F32, name=f"{uniq}_mask", tag="mg_mask")
    nc.vector.tensor_scalar(out=mask[:, :width], in0=m2[:, :width],
                            scalar1=-2e38, scalar2=None, op0=AL.is_le)
    csA = small.tile([128, NG], F32, name=f"{uniq}_csA", tag="mg_csA")
    csB = small.tile([128, NG], F32, name=f"{uniq}_csB", tag="mg_csB")
    nc.vector.tensor_copy(csA[:, :width], mask[:, :width])
    src, dst = csA, csB
    sh = 1
    while sh < width:
        nc.vector.tensor_copy(dst[:, 0:sh], src[:, 0:sh])
        nc.vector.tensor_tensor(out=dst[:, sh:width], in0=src[:, sh:width],
                                in1=src[:, 0:width - sh], op=AL.add)
        src, dst = dst, src
        sh *= 2
    rk = small.tile([128, NG], F32, name=f"{uniq}_rk", tag="mg_rk")
    nc.vector.tensor_tensor(out=rk[:, :width], in0=src[:, :width],
                            in1=mask[:, :width], op=AL.mult)
    nc.vector.tensor_scalar(out=rk[:, :width], in0=rk[:, :width], scalar1=-1.0,
                            scalar2=None, op0=AL.add)
    rk16 = small.tile([128, NG], I16, name=f"{uniq}_rk16", tag="mg_rk16")
    nc.vector.tensor_copy(rk16[:, :width], rk[:, :width])
    for plane, out16 in payloads:
        nc.gpsimd.local_scatter(out16[:].bitcast(I16), plane.bitcast(I16),
                                rk16[:, :width], channels=128, num_elems=K,
                                num_idxs=width)


def build():
    nc = bacc.Bacc("TRN2", target_bir_lowering=False, num_devices=NCORES)

    xh_in = nc.dram_tensor("xh", [4, 128, B], U16, kind="ExternalInput")
    xc8_in = nc.dram_tensor("xc8", [4, 128, 2, B], U8, kind="ExternalInput")
    yh_in = nc.dram_tensor("yh", [4, 128, COLS], U16, kind="ExternalInput")
    yc8_in = nc.dram_tensor("yc8", [4, 128, 2, COLS], U8, kind="ExternalInput")
    yn3_in = nc.dram_tensor("yn3", [3, COLS], U16, kind="ExternalInput")
    xn_in = nc.dram_tensor("xn", [128, QTILES], F32, kind="ExternalInput")
    bnd_in = nc.dram_tensor("bnd", [1, NCLASS], F32, kind="ExternalInput")
    out_d = nc.dram_tensor("out", [2 * 128, NCLASS], F32,
                           kind="ExternalOutput")

    ag_in = [nc.dram_tensor(f"ag_in{g}", [GQT * 128, 2 * K], F32)
             for g in range(GROUPS)]
    ag_out = nc.dram_tensor("ag_out", [GROUPS * NCORES * GQT * 128, 2 * K],
                            F32, addr_space="Shared")

    with tile.TileContext(nc) as tc:
        with tc.tile_pool(name="res", bufs=1) as res, \
             tc.tile_pool(name="zps", bufs=2, space="PSUM") as zps, \
             tc.tile_pool(name="candp", bufs=2) as candp, \
             tc.tile_pool(name="small", bufs=2) as small:

            # ---------------- resident tensors ----------------
            xh_r = [res.tile([128, B], F16, name=f"xh{k}") for k in range(4)]
            xc8_r = [res.tile([128, 2, B], F8E4, name=f"xc{k}")
                     for k in range(4)]
            yh_r = [res.tile([128, COLS], F16, name=f"yh{k}") for k in range(4)]
            yc8_r = [res.tile([128, 2, COLS], F8E4, name=f"yc{k}")
                     for k in range(4)]
            yn3 = res.tile([3, COLS], BF16)
            ones3 = res.tile([3, 128], BF16)
            xn_r = res.tile([128, QTILES], F32)
            bnd_f = res.tile([128, NCLASS], F32)
            cio_f = res.tile([128, NCLASS], F32)
            base32 = res.tile([128, NCAND], U16)

            # small constants first
            nc.vector.memset(ones3[:], -1.0)
            nc.sync.dma_start(yn3[:].bitcast(U16), yn3_in[:])
            nc.sync.dma_start(xn_r[:], xn_in[:])
            nc.gpsimd.iota(base32[:], pattern=[[2048, NW], [0, 8]],
                           channel_multiplier=0)
            cio_i = small.tile([128, NCLASS], I32, tag="cioi", bufs=1)
            nc.gpsimd.iota(cio_i[:], pattern=[[1, NCLASS]],
                           channel_multiplier=0)
            nc.vector.tensor_copy(cio_f[:], cio_i[:])
            # broadcast bnd row to 128 partitions via f32 matmul
            bnd_row = small.tile([1, NCLASS], F32, tag="bndrow", bufs=1)
            nc.sync.dma_start(bnd_row[:], bnd_in[:])
            ones1 = small.tile([1, 128], F32, tag="ones1", bufs=1)
            nc.vector.memset(ones1[:], 1.0)
            bps = zps.tile([128, 2048], F32, name="bps", tag="zw")
            nc.tensor.matmul(bps[:, 0:NCLASS], ones1[:], bnd_row[:],
                             start=True, stop=True)
            nc.scalar.copy(bnd_f[:], bps[:, 0:NCLASS])

            # x side (needed by every window)
            for k in range(4):
                nc.sync.dma_start(xh_r[k][:].bitcast(U16), xh_in[k])
                nc.sync.dma_start(xc8_r[k][:].bitcast(U8), xc8_in[k])
            # y side in column blocks so window 0 can start early
            for (a, b) in WIN:
                for k in range(4):
                    nc.sync.dma_start(yh_r[k][:, a:b].bitcast(U16),
                                      yh_in[k, :, a:b])
                    nc.sync.dma_start(yc8_r[k][:, :, a:b].bitcast(U8),
                                      yc8_in[k, :, :, a:b])

            pid_sp = nc.sync.partition_id()

            def global_phase(l):
                """Merge + vote for owned qtile qt = pid + 8*l."""
                gv = small.tile([128, NG], F32, name=f"gv{l}", tag="gv")
                gl = small.tile([128, NG], F32, name=f"gl{l}", tag="gl")
                gvl3 = small.tile([128, NCORES, 2 * K], F32,
                                  name=f"gvl{l}", tag="gvl")
                g_reg = (pid_sp + 8 * l) // 4
                lq_reg = pid_sp % 4
                for c2 in range(NCORES):
                    base = g_reg * (NCORES * GQT * 128) + c2 * (GQT * 128)
                    nc.sync.dma_start(
                        gvl3[:, c2, :],
                        ag_out[bass.ds(base + lq_reg * 128, 128), :])
                nc.vector.tensor_copy(
                    gv[:].rearrange("p (c k) -> p c k", c=NCORES),
                    gvl3[:, :, 0:K])
                nc.vector.tensor_copy(
                    gl[:].rearrange("p (c k) -> p c k", c=NCORES),
                    gvl3[:, :, K:2 * K])
                vlo = small.tile([128, NG], U16, name=f"gvlo{l}", tag="vlo")
                vhi = small.tile([128, NG], U16, name=f"gvhi{l}", tag="vhi")
                gvu = gv[:].bitcast(U16).rearrange("p (a two) -> p a two",
                                                   two=2)
                nc.vector.tensor_copy(vlo[:], gvu[:, :, 0:1])
                nc.vector.tensor_copy(vhi[:], gvu[:, :, 1:2])
                glu = small.tile([128, NG], U16, name=f"glu{l}", tag="glu")
                nc.vector.tensor_copy(glu[:], gl[:])
                slo = small.tile([128, K], U16, name=f"gslo{l}", tag="slo16")
                shi = small.tile([128, K], U16, name=f"gshi{l}", tag="shi16")
                sla = small.tile([128, K], U16, name=f"gsla{l}", tag="sgi16")
                _merge_top16(nc, small, f"gm{l}", gv, NG,
                             [(vlo[:], slo), (vhi[:], shi), (glu[:], sla)])
                v16 = small.tile([128, K], F32, name=f"gv16{l}", tag="v16")
                v16u = v16[:].bitcast(U16).rearrange("p (a two) -> p a two",
                                                     two=2)
                nc.vector.tensor_copy(v16u[:, :, 0:1], slo[:])
                nc.vector.tensor_copy(v16u[:, :, 1:2], shi[:])
                lab16 = small.tile([128, K], F32, name=f"glab{l}",
                                   tag="lab16")
                nc.vector.tensor_copy(lab16[:], sla[:])
                xn_col = small.tile([128, 1], F32, name=f"xnc{l}",
                                    tag="xncol")
                nc.sync.dma_start(xn_col[:],
                                  xn_r[:, bass.ds(pid_sp + 8 * l, 1)])
                dsq = small.tile([128, K], F32, name=f"dsq{l}", tag="dsq")
                nc.scalar.activation(dsq[:], v16[:], AF.Sqrt,
                                     scale=-1.0 / BETA, bias=xn_col[:, 0:1])
                ew = small.tile([128, K], F32, name=f"ew{l}", tag="ew")
                zsum = small.tile([128, 1], F32, name=f"zs{l}", tag="zs")
                nc.scalar.activation(ew[:], dsq[:], AF.Exp, scale=-1.0,
                                     accum_out=zsum[:, 0:1])
                rz = small.tile([128, 1], F32, name=f"rz{l}", tag="rz")
                nc.vector.reciprocal(rz[:], zsum[:])
                wt = small.tile([128, K], F32, name=f"wt{l}", tag="wt")
                nc.vector.tensor_scalar(out=wt[:], in0=ew[:],
                                        scalar1=rz[:, 0:1], scalar2=None,
                                        op0=AL.mult)
                vote = small.tile([128, NCLASS], F32, name=f"vote{l}",
                                  tag="vote")
                voteg = small.tile([128, NCLASS], F32, name=f"voteg{l}",
                                   tag="voteg")
                tmp = small.tile([128, NCLASS], F32, name=f"vtmp{l}",
                                 tag="vtmp")
                tmpg = small.tile([128, NCLASS], F32, name=f"vtmpg{l}",
                                  tag="vtmpg")
                nc.vector.memset(vote[:], 0.0)
                nc.gpsimd.memset(voteg[:], 0.0)
                for r in range(K // 2):
                    nc.vector.tensor_scalar(out=tmp[:], in0=cio_f[:],
                                            scalar1=lab16[:, r:r + 1],
                                            scalar2=wt[:, r:r + 1],
                                            op0=AL.is_equal, op1=AL.mult)
                    nc.vector.tensor_tensor(out=vote[:], in0=vote[:],
                                            in1=tmp[:], op=AL.add)
                    r2 = r + K // 2
                    nc.gpsimd.tensor_scalar(out=tmpg[:], in0=cio_f[:],
                                            scalar1=lab16[:, r2:r2 + 1],
                                            scalar2=wt[:, r2:r2 + 1],
                                            op0=AL.is_equal, op1=AL.mult)
                    nc.gpsimd.tensor_tensor(out=voteg[:], in0=voteg[:],
                                            in1=tmpg[:], op=AL.add)
                nc.vector.tensor_tensor(out=vote[:], in0=vote[:],
                                        in1=voteg[:], op=AL.add)
                nc.sync.dma_start(out_d[l * 128:(l + 1) * 128, :], vote[:])

            # ---------------- main loop ----------------
            for g in range(GROUPS):
                cands = {}
                for lq in range(GQT):
                    cands[lq] = (
                        candp.tile([128, NCAND], F32, name=f"cv{g}_{lq}",
                                   tag=f"cv{lq}"),
                        candp.tile([128, NCAND], U16, name=f"ci{g}_{lq}",
                                   tag=f"ci{lq}"),
                    )
                for w, (a, b) in enumerate(WIN):
                    wlen = b - a
                    for lq in range(GQT):
                        qt = g * GQT + lq
                        qs = qt * 128
                        ps = zps.tile([128, 2048], F32,
                                      name=f"ps{g}_{w}_{lq}", tag="zw")
                        for co0 in range(0, wlen, 512):
                            co = a + co0
                            cw = min(512, b - co)
                            pslice = ps[:, co0:co0 + cw]
                            nc.tensor.matmul(pslice, ones3[:],
                                             yn3[:, co:co + cw],
                                             start=True, stop=False)
                            for k in range(4):
                                nc.tensor.matmul(pslice,
                                                 xh_r[k][:, qs:qs + 128],
                                                 yh_r[k][:, co:co + cw],
                                                 start=False, stop=False)
                            for k in range(4):
                                nc.tensor.matmul(
                                    pslice,
                                    xc8_r[k][:, :, qs:qs + 128],
                                    yc8_r[k][:, :, co:co + cw],
                                    start=False, stop=(k == 3),
                                    perf_mode=DR)
                        cv, ci = cands[lq]
                        nc.vector.max(cv[:, w * 8:w * 8 + 8], ps[:, :wlen])
                        nc.vector.max_index(ci[:, w * 8:w * 8 + 8],
                                            cv[:, w * 8:w * 8 + 8],
                                            ps[:, :wlen])

                # owned qtile qt=pid sits in group 0 or 1; AG_1 has had all
                # of group 2's scans to land, so this inserts no engine wait
                if g == 2:
                    global_phase(0)

                for lq in range(GQT):
                    qt = g * GQT + lq
                    cv, ci = cands[lq]
                    gi = small.tile([128, NCAND], U16, name=f"gi{qt}",
                                    tag="gi")
                    nc.vector.tensor_tensor(out=gi[:], in0=ci[:],
                                            in1=base32[:], op=AL.add)
                    vlo = small.tile([128, NCAND], U16, name=f"vlo{qt}",
                                     tag="vlo")
                    vhi = small.tile([128, NCAND], U16, name=f"vhi{qt}",
                                     tag="vhi")
                    cvu = cv[:].bitcast(U16).rearrange(
                        "p (a two) -> p a two", two=2)
                    nc.vector.tensor_copy(vlo[:, :NCAND], cvu[:, :, 0:1])
                    nc.vector.tensor_copy(vhi[:, :NCAND], cvu[:, :, 1:2])
                    slo = small.tile([128, K], U16, name=f"slo16_{qt}",
                                     tag="slo16")
                    shi = small.tile([128, K], U16, name=f"shi16_{qt}",
                                     tag="shi16")
                    sgi = small.tile([128, K], U16, name=f"sgi16_{qt}",
                                     tag="sgi16")
                    _merge_top16(nc, small, f"lm{qt}", cv, NCAND,
                                 [(vlo[:, :NCAND], slo),
                                  (vhi[:, :NCAND], shi), (gi[:], sgi)])
                    v16 = small.tile([128, K], F32, name=f"v16_{qt}",
                                     tag="v16")
                    v16u = v16[:].bitcast(U16).rearrange(
                        "p (a two) -> p a two", two=2)
                    nc.vector.tensor_copy(v16u[:, :, 0:1], slo[:])
                    nc.vector.tensor_copy(v16u[:, :, 1:2], shi[:])
                    # labels: sum of sign(idx + 0.5 - bnd_k) = 2*label-98
                    gfh = small.tile([128, K], F32, name=f"gfh{qt}",
                                     tag="gfh")
                    nc.vector.tensor_scalar(out=gfh[:], in0=sgi[:],
                                            scalar1=0.5, scalar2=None,
                                            op0=AL.add)
                    junk = small.tile([128, NCLASS], F32, name=f"sj{qt}",
                                      tag="sjunk")
                    labsum = small.tile([128, K], F32, name=f"ls{qt}",
                                        tag="labsum")
                    for r in range(K):
                        nc.scalar.activation(junk[:], bnd_f[:], AF.Sign,
                                             bias=gfh[:, r:r + 1],
                                             scale=-1.0,
                                             accum_out=labsum[:, r:r + 1])
                    lab16 = small.tile([128, K], F32, name=f"lab{qt}",
                                       tag="lab16l")
                    nc.vector.tensor_scalar(out=lab16[:], in0=labsum[:],
                                            scalar1=0.5, scalar2=49.0,
                                            op0=AL.mult, op1=AL.add)
                    nc.sync.dma_start(
                        ag_in[g][lq * 128:(lq + 1) * 128, 0:K], v16[:])
                    nc.sync.dma_start(
                        ag_in[g][lq * 128:(lq + 1) * 128, K:2 * K], lab16[:])

                nc.gpsimd.collective_compute(
                    "AllGather", AL.bypass,
                    replica_groups=[list(range(NCORES))],
                    ins=[ag_in[g][:].opt()],
                    outs=[ag_out[g * NCORES * GQT * 128:
                                 (g + 1) * NCORES * GQT * 128, :].opt()])

                if g == 3:
                    global_phase(1)

    nc.finalize()
    return nc


_NC_CACHE = None


def _e4m3(a):
    return np.clip(a, -240.0, 240.0).astype(ml_dtypes.float8_e4m3fn)


def _prep_host(x, tf, tl):
    """Build per-core input maps (host-side marshalling)."""
    x = np.ascontiguousarray(np.asarray(x, dtype=np.float32))
    tf = np.ascontiguousarray(np.asarray(tf, dtype=np.float32))
    tl = np.asarray(tl, dtype=np.int64)

    X = 1024.0 * x
    Xh16 = X.astype(np.float16)
    Xh = Xh16.astype(np.float32)
    Xl = X - Xh
    A_lhs = _e4m3(Xh / 512.0)
    B_lhs = _e4m3(Xl)

    xh_t = np.ascontiguousarray(
        Xh16.T.reshape(4, 128, B).view(np.uint16))
    xc8 = np.ascontiguousarray(np.stack(
        [A_lhs.T.reshape(4, 128, B).view(np.uint8),
         B_lhs.T.reshape(4, 128, B).view(np.uint8)],
        axis=2))

    xn = (x * x).sum(1).astype(np.float32).reshape(QTILES, 128).T
    xn = np.ascontiguousarray(xn)

    in_maps = []
    for c in range(NCORES):
        sl = slice(c * NSHARD, (c + 1) * NSHARD)
        labs = tl[sl]
        feats = tf[sl]
        perm = np.argsort(labs, kind="stable")
        feats = np.ascontiguousarray(feats[perm])
        labs_s = labs[perm]
        bnd = np.searchsorted(labs_s, np.arange(NCLASS), side="left")

        y = np.zeros((COLS, D), np.float32)
        y[:NSHARD] = feats
        yn = np.full(COLS, BIG, np.float32)
        yn[:NSHARD] = (feats * feats).sum(1)

        Yh16 = y.astype(np.float16)
        Yh = Yh16.astype(np.float32)
        Yl = y - Yh
        A_rhs = _e4m3(512.0 * Yl)
        B_rhs = _e4m3(Yh)

        yh_t = np.ascontiguousarray(
            Yh16.T.reshape(4, 128, COLS).view(np.uint16))
        yc8 = np.ascontiguousarray(np.stack(
            [A_rhs.T.reshape(4, 128, COLS).view(np.uint8),
             B_rhs.T.reshape(4, 128, COLS).view(np.uint8)],
            axis=2))

        v = (BETA * yn).astype(np.float32)
        y1 = v.astype(ml_dtypes.bfloat16)
        y2 = (v - y1.astype(np.float32)).astype(ml_dtypes.bfloat16)
        y3 = (v - y1.astype(np.float32) - y2.astype(np.float32)).astype(
            ml_dtypes.bfloat16)
        yn3 = np.ascontiguousarray(
            np.stack([y1, y2, y3], axis=0).view(np.uint16))

        in_maps.append({
            "xh": xh_t,
            "xc8": xc8,
            "yh": yh_t,
            "yc8": yc8,
            "yn3": yn3,
            "xn": xn,
            "bnd": bnd.astype(np.float32)[None, :],
        })
    return in_maps


def kernel(x, train_features, train_labels, **run_kwargs):
    global _NC_CACHE
    in_maps = _prep_host(x, train_features, train_labels)
    if _NC_CACHE is None:
        _NC_CACHE = build()
    res = bass_utils.run_bass_kernel_spmd(
        _NC_CACHE, in_maps, core_ids=list(range(NCORES)), **run_kwargs)
    global LAST_RESULTS
    LAST_RESULTS = res
    out = np.zeros((B, NCLASS), np.float32)
    for c in range(NCORES):
        o = res.results[c]["out"]
        out[c * 128:(c + 1) * 128] = o[0:128]
        out[(8 + c) * 128:(9 + c) * 128] = o[128:256]
    return out.astype(np.float32)


LAST_RESULTS = None
